# revision 28
# baseline (speedup 1.0000x reference)
"""Multi-head causal attention (B=2, S=2048, D=2048, 16 heads) on 8 TRN2 cores.

Sharding: 2-way batch parallel x 4-way head tensor-parallel (4 heads/core).
Each core computes q/k/v projections for its 4 heads, causal softmax
attention, and a partial o-projection; the host sums the 4 partials per batch.

Host pre-transposes x and the weight slices so every on-chip matmul has its
contraction dim on SBUF partitions (no on-chip transposes at all):
  xT  [D, S]   = x[b].T
  wqT [D, JC]  = wq[j0:j0+512, :].T     (same wkT, wvT)
  woT [JC, D]  = wo[:, j0:j0+512].T

On-chip dataflow (per core), all matmuls in float32r (FP22 multiply,
fp32 accumulate in PSUM):
  phase 1: kT[j,s] and v[s,dv] stay resident in SBUF; qT[j,s] spills to DRAM
           (read back in 256KB chunks). wq/wk resident; wv streamed per
           dt-tile (dt-outer loop, 4 parallel PSUM banks).
  phase 2 (per head, per 512-wide i-chunk):
           a) scoresT[j,i] = k_h @ q_h.T per j-tile, exp on ScalarE (scale
              fused), diagonal j-tiles trimmed to i >= j and masked with a
              triangular 128x128 mask;
           b) denominator: ones[128,128] @ probsT accumulated over j-tiles
              (every PSUM partition gets the column sum - broadcast built in);
           c) attT[dv,i] += v_h[j,dv]-stationary @ probsT[j,i] over j-tiles;
           normalize with DVE reciprocal + multiply.
  phase 3: out[s,m] partial = sum_h attT_h.T @ woT_h, DMA'd to DRAM.
"""

import math

import numpy as np

B, S, D = 2, 2048, 2048
HEADS, HEAD_DIM = 16, 128
P = 128
JC = 512          # per-core projection width (4 heads x 128)
SC = 512          # s-chunk / matmul moving width
DT = D // P       # 16 contraction tiles
NSC = S // SC     # 4 s-chunks
NST = S // P      # 16 s-tiles
HPC = 4           # heads per core
N_CORES = 8
SCALE = 1.0 / math.sqrt(HEAD_DIM)

_NC_CACHE = {}


def build_module(reps=1, phases=(1, 2, 3)):
    """Build + compile the (single-program SPMD) Bass module once.

    reps>1 repeats the whole kernel body inside one NEFF (for timing:
    differencing per-call wall times cancels the fixed dispatch overhead).
    phases: which kernel phases to include (timing experiments only).
    """
    phases = tuple(phases)
    key = (reps, phases)
    if key in _NC_CACHE:
        return _NC_CACHE[key]

    from contextlib import ExitStack

    import concourse.tile as tile
    from concourse import bacc
    import concourse.mybir as mybir

    f32r = mybir.dt.float32r
    f32 = mybir.dt.float32
    FT = mybir.ActivationFunctionType

    nc = bacc.Bacc(
        "TRN2", target_bir_lowering=False, debug=False, num_devices=N_CORES
    )

    xT = nc.dram_tensor("xT", [D, S], f32r, kind="ExternalInput").ap()
    wqT = nc.dram_tensor("wqT", [D, JC], f32r, kind="ExternalInput").ap()
    wkT = nc.dram_tensor("wkT", [D, JC], f32r, kind="ExternalInput").ap()
    wvT = nc.dram_tensor("wvT", [D, JC], f32r, kind="ExternalInput").ap()
    woT = nc.dram_tensor("woT", [JC, D], f32r, kind="ExternalInput").ap()
    # mask[j, c] = 1 iff j <= c : causal triangle for a diagonal 128-block
    mask = nc.dram_tensor("mask", [P, P], f32r, kind="ExternalInput").ap()
    ones = nc.dram_tensor("ones", [P, P], f32r, kind="ExternalInput").ap()
    out = nc.dram_tensor("out", [S, D], f32, kind="ExternalOutput").ap()

    qTd = nc.dram_tensor("qTd", [JC, S], f32r, kind="Internal").ap()

    with tile.TileContext(nc) as tc, ExitStack() as ctx:
        consts = ctx.enter_context(tc.tile_pool(name="consts", bufs=1))
        stage = ctx.enter_context(tc.tile_pool(name="stage", bufs=4))

        mask_sb = consts.tile([P, P], f32r, tag="mask", name="mask_sb")
        nc.sync.dma_start(mask_sb, mask)
        ones_sb = consts.tile([P, P], f32r, tag="ones", name="ones_sb")
        nc.sync.dma_start(ones_sb, ones)

        xT_r = xT.rearrange("(dt p) s -> p dt s", p=P)
        wqT_r = wqT.rearrange("(dt p) j -> p dt j", p=P)
        wkT_r = wkT.rearrange("(dt p) j -> p dt j", p=P)

        for _rep in range(reps):
            with ExitStack() as prep:
                # kT_all [p, head, s] and v_all [p, s-tile, dv] stay resident
                # across phases 1-2 (64KB/partition)
                kvpool = prep.enter_context(tc.tile_pool(name="kvpool", bufs=1))
                kT_all = kvpool.tile([P, HPC, S], f32r, tag="kT", name="kT_all")
                v_all = kvpool.tile([P, NST, JC], f32r, tag="v", name="v_all")

                # ---------- Phase 1: q/k/v projections ----------
                with ExitStack() as p1:
                    wpool = p1.enter_context(tc.tile_pool(name="wpool", bufs=1))
                    wvpool = p1.enter_context(
                        tc.tile_pool(name="wvpool", bufs=3)
                    )
                    xpool = p1.enter_context(tc.tile_pool(name="xpool", bufs=2))
                    psum1 = p1.enter_context(
                        tc.tile_pool(name="psum1", bufs=2, space="PSUM")
                    )


                    # wq/wk resident; DMAs split per-4-dt and emitted after the
                    # first chunk's x/wv DMAs so the v projection starts early
                    wq_sb = wpool.tile([P, DT, JC], f32r, tag="wq", name="wq_sb")
                    wk_sb = wpool.tile([P, DT, JC], f32r, tag="wk", name="wk_sb")

                    for sc in range(NSC):
                        xc = xpool.tile(
                            [P, DT, SC], f32r, tag="x", name=f"xc_{sc}"
                        )
                        for q4 in range(4):
                            dts = slice(q4 * 4, (q4 + 1) * 4)
                            nc.sync.dma_start(
                                xc[:, dts, :],
                                xT_r[:, dts, sc * SC:(sc + 1) * SC],
                            )

                        # v projection first (dt-outer; wv streamed per dt):
                        # PE can start after ~1.5MB of DMA
                        ps_v = [
                            psum1.tile(
                                [P, JC], f32, tag=f"pv{t}", bufs=1, name="ps_v"
                            )
                            for t in range(4)
                        ]
                        for dt in range(DT):
                            wv_dt = wvpool.tile(
                                [P, JC], f32r, tag="wv", name="wv_dt"
                            )
                            nc.scalar.dma_start(
                                wv_dt, wvT[dt * P:(dt + 1) * P, :]
                            )
                            for t in range(4):
                                # v tile [s, dv] = x_chunk.T @ wv_slice
                                nc.tensor.matmul(
                                    ps_v[t],
                                    lhsT=xc[:, dt, t * P:(t + 1) * P],
                                    rhs=wv_dt,
                                    start=(dt == 0),
                                    stop=(dt == DT - 1),
                                )
                        for t in range(4):
                            nc.vector.tensor_copy(
                                v_all[:, sc * 4 + t, :], ps_v[t]
                            )
                        if sc == 0:
                            # now queue the resident k/q weights (k first:
                            # it's the next consumer)
                            for q4 in range(4):
                                dts = slice(q4 * 4, (q4 + 1) * 4)
                                nc.scalar.dma_start(
                                    wk_sb[:, dts, :], wkT_r[:, dts, :]
                                )
                            for q4 in range(4):
                                dts = slice(q4 * 4, (q4 + 1) * 4)
                                nc.scalar.dma_start(
                                    wq_sb[:, dts, :], wqT_r[:, dts, :]
                                )

                        # k projection into resident kT_all
                        for t in range(4):
                            ps = psum1.tile([P, SC], f32, tag="pj", name="ps_k")
                            for dt in range(DT):
                                nc.tensor.matmul(
                                    ps,
                                    lhsT=wk_sb[:, dt, t * P:(t + 1) * P],
                                    rhs=xc[:, dt, :],
                                    start=(dt == 0),
                                    stop=(dt == DT - 1),
                                )
                            nc.vector.tensor_copy(
                                kT_all[:, t, sc * SC:(sc + 1) * SC], ps
                            )

                        # q projection spilled to DRAM
                        for t in range(4):
                            ps = psum1.tile([P, SC], f32, tag="pj", name="ps_q")
                            for dt in range(DT):
                                nc.tensor.matmul(
                                    ps,
                                    lhsT=wq_sb[:, dt, t * P:(t + 1) * P],
                                    rhs=xc[:, dt, :],
                                    start=(dt == 0),
                                    stop=(dt == DT - 1),
                                )
                            stg = stage.tile(
                                [P, SC], f32r, tag="stage", name="stg_q"
                            )
                            nc.vector.tensor_copy(stg, ps)
                            nc.scalar.dma_start(
                                qTd[t * P:(t + 1) * P, sc * SC:(sc + 1) * SC],
                                stg,
                            )

                # ---------- Phase 2/3 pools ----------
                if 2 not in phases:
                    continue
                with ExitStack() as p2:
                    opool = p2.enter_context(tc.tile_pool(name="opool", bufs=1))
                    attp = p2.enter_context(tc.tile_pool(name="attp", bufs=1))
                    qpool = p2.enter_context(tc.tile_pool(name="qpool", bufs=3))
                    ppool = p2.enter_context(tc.tile_pool(name="ppool", bufs=9))
                    rpool = p2.enter_context(tc.tile_pool(name="rpool", bufs=2))
                    ostage = p2.enter_context(
                        tc.tile_pool(name="ostage", bufs=2)
                    )
                    psum2 = p2.enter_context(
                        tc.tile_pool(name="psum2", bufs=2, space="PSUM")
                    )

                    # Phase 3 weights (prefetch during phase 2)
                    woTs = opool.tile([P, HPC, D], f32r, tag="wo", name="woTs")
                    nc.sync.dma_start(
                        woTs, woT.rearrange("(hh p) m -> p hh m", p=P)
                    )

                    # ---------- Phase 2: causal attention per head ----------
                    attTs = []
                    for h in range(HPC):
                        attT = attp.tile(
                            [P, S], f32r, tag=f"attT{h}", name=f"attT_{h}"
                        )
                        attTs.append(attT)

                        for ic in range(NSC):
                            njt = 4 * ic + 4  # causal: j-tiles 0..njt-1
                            offs = [
                                max(0, (jt - 4 * ic) * P) for jt in range(njt)
                            ]
                            qT_sb = qpool.tile(
                                [P, SC], f32r, tag="qT", name="qT_c"
                            )
                            nc.sync.dma_start(
                                qT_sb,
                                qTd[
                                    h * P:(h + 1) * P, ic * SC:(ic + 1) * SC
                                ],
                            )

                            # a) scoresT into 2-bank psum regions
                            # (pairs of j-tiles), ONE exp per pair - halves
                            # the ScalarE per-instruction overhead
                            pts = []  # (wide tile, half index u) per jt
                            for g in range(njt // 2):
                                ps_s = psum2.tile(
                                    [P, 2 * SC], f32, tag="score", bufs=2,
                                    name="ps_s",
                                )
                                ptw = ppool.tile(
                                    [P, 2 * SC], f32r, tag="prob", name="pt"
                                )
                                off0 = offs[2 * g]
                                for u in range(2):
                                    jt = 2 * g + u
                                    pts.append((ptw, u))
                                    # write from the pair's min offset so the
                                    # single wide exp never reads unwritten
                                    # psum; den/pv still slice from offs[jt]
                                    nc.tensor.matmul(
                                        ps_s[:, u * SC + off0:(u + 1) * SC],
                                        lhsT=kT_all[
                                            :, h, jt * P:(jt + 1) * P
                                        ],
                                        rhs=qT_sb[:, off0:],
                                        start=True,
                                        stop=True,
                                    )
                                if off0 == 0:
                                    nc.scalar.activation(
                                        ptw, ps_s, FT.Exp, scale=SCALE
                                    )
                                else:
                                    # columns [SC, SC+off0) are unwritten;
                                    # exp each half separately
                                    nc.scalar.activation(
                                        ptw[:, off0:SC], ps_s[:, off0:SC],
                                        FT.Exp, scale=SCALE,
                                    )
                                    nc.scalar.activation(
                                        ptw[:, SC + off0:],
                                        ps_s[:, SC + off0:],
                                        FT.Exp, scale=SCALE,
                                    )
                                for u in range(2):
                                    jt = 2 * g + u
                                    if jt >= 4 * ic:
                                        # triangular mask on diagonal block
                                        off = offs[jt]
                                        nc.vector.tensor_mul(
                                            out=ptw[
                                                :,
                                                u * SC + off:u * SC + off + P,
                                            ],
                                            in0=ptw[
                                                :,
                                                u * SC + off:u * SC + off + P,
                                            ],
                                            in1=mask_sb,
                                        )

                            # b) denominator (ones stationary loaded once)
                            ps_den = psum2.tile(
                                [P, SC], f32, tag="den", name="ps_den"
                            )
                            for jt in range(njt):
                                off = offs[jt]
                                ptw, u = pts[jt]
                                nc.tensor.matmul(
                                    ps_den[:, off:],
                                    lhsT=ones_sb,
                                    rhs=ptw[:, u * SC + off:(u + 1) * SC],
                                    start=(jt == 0),
                                    stop=(jt == njt - 1),
                                    skip_group_check=True,
                                )
                            rec = rpool.tile(
                                [P, SC], f32, tag="rec", name="rec"
                            )
                            nc.vector.reciprocal(rec, ps_den)

                            # c) attT[dv, i] += v_h[j, dv] stationary @ probsT
                            ps_pv = psum2.tile(
                                [P, SC], f32, tag="pv", name="ps_pv"
                            )
                            for jt in range(njt):
                                off = offs[jt]
                                ptw, u = pts[jt]
                                nc.tensor.matmul(
                                    ps_pv[:, off:],
                                    lhsT=v_all[
                                        :, jt, h * HEAD_DIM:(h + 1) * HEAD_DIM
                                    ],
                                    rhs=ptw[:, u * SC + off:(u + 1) * SC],
                                    start=(jt == 0),
                                    stop=(jt == njt - 1),
                                    skip_group_check=True,
                                )
                            nc.vector.tensor_mul(
                                out=attT[:, ic * SC:(ic + 1) * SC],
                                in0=ps_pv,
                                in1=rec,
                            )

                    # ---------- Phase 3: partial o-projection ----------
                    for st in range(NST if 3 in phases else 0):
                        og = ostage.tile([P, D], f32, tag="og", name="og")
                        for mc in range(D // SC):
                            # den/pv slots are idle during o-proj
                            ps = psum2.tile(
                                [P, SC], f32,
                                tag=("den" if mc % 2 == 0 else "pv"),
                                name="ps_o",
                            )
                            for hh in range(HPC):
                                nc.tensor.matmul(
                                    ps,
                                    lhsT=attTs[hh][:, st * P:(st + 1) * P],
                                    rhs=woTs[:, hh, mc * SC:(mc + 1) * SC],
                                    start=(hh == 0),
                                    stop=(hh == HPC - 1),
                                )
                            nc.vector.tensor_copy(
                                og[:, mc * SC:(mc + 1) * SC], ps
                            )
                        nc.sync.dma_start(out[st * P:(st + 1) * P, :], og)

    nc.compile()
    _NC_CACHE[key] = nc
    return nc


def make_in_maps(x, wq, wk, wv, wo):
    x = np.asarray(x, dtype=np.float32)
    wq = np.asarray(wq, dtype=np.float32)
    wk = np.asarray(wk, dtype=np.float32)
    wv = np.asarray(wv, dtype=np.float32)
    wo = np.asarray(wo, dtype=np.float32)
    # mask[j, c] = 1 iff key j visible to query c within a diagonal block
    causal = np.triu(np.ones((P, P), dtype=np.float32))
    ones = np.ones((P, P), dtype=np.float32)
    in_maps = []
    for c in range(N_CORES):
        b, g = divmod(c, HPC)
        j0 = g * JC
        in_maps.append(
            {
                "xT": np.ascontiguousarray(x[b].T),
                "wqT": np.ascontiguousarray(wq[j0:j0 + JC].T),
                "wkT": np.ascontiguousarray(wk[j0:j0 + JC].T),
                "wvT": np.ascontiguousarray(wv[j0:j0 + JC].T),
                "woT": np.ascontiguousarray(wo[:, j0:j0 + JC].T),
                "mask": causal,
                "ones": ones,
            }
        )
    return in_maps


def combine_outputs(results):
    out = np.zeros((B, S, D), dtype=np.float32)
    for c in range(N_CORES):
        out[c // HPC] += results[c]["out"]
    return out


def kernel(x, wq, wk, wv, wo):
    from concourse.bass_utils import run_bass_kernel_spmd

    nc = build_module()
    in_maps = make_in_maps(x, wq, wk, wv, wo)
    res = run_bass_kernel_spmd(nc, in_maps, list(range(N_CORES)))
    return combine_outputs(res.results)
